# revision 18
# baseline (speedup 1.0000x reference)
"""Causal GQA self-attention (B=2, S=2048, D=2048, H=16, KV=4) on 8 TRN2 cores.

Sharding: core = (b, g) with b = batch (2) x g = kv-head group (4).
Each core computes 4 q-heads / 1 kv-head for one batch and a partial
projection output [S, D]; host sums the 4 group partials per batch.

v3: all-bf16 matmuls, per-t merged loop with proj of t-1 interleaved
after attention t, host-preswizzled x (contiguous DMA), q-norm via DVE
bn_stats, q norm-scale folded into the transpose as a diag matmul,
k norm-scale folded into exp's per-partition scale, rope add folded
into psum-accumulated transpose pair, proj written psum->DRAM directly.
"""
import os
import sys

if '/opt/trn_rl_repo' not in sys.path:
    sys.path.insert(0, '/opt/trn_rl_repo')

import numpy as np
import ml_dtypes

BF = ml_dtypes.bfloat16

B, S, D = 2, 2048, 2048
NH_TOT, NKV_TOT, HD = 16, 4, 128
NH = 4                 # q heads per core
NT = S // 128          # 16 s-tiles
NC_ = D // 128         # 16 c-tiles
T = 4                  # q-slices of 512
SM = 1.0 / np.sqrt(HD)
EPS = float(np.finfo(np.float32).eps)
ROPE_BASE = 10000.0
HORD = (2, 3, 0, 1)    # head emission order (attention runs pair hp=2 first)

_PROG = None


def _build_program():
    import concourse.bass as bass
    import concourse.mybir as mybir
    import concourse.tile as tile
    from concourse import bacc
    from concourse.alu_op_type import AluOpType

    F32 = mybir.dt.float32
    BF16 = mybir.dt.bfloat16
    AF = mybir.ActivationFunctionType

    nc = bacc.Bacc("TRN2", target_bir_lowering=False, debug=False)

    XTS = nc.dram_tensor("XTS", [128, NT, NC_, 128], BF16, kind="ExternalInput")
    WQKV = nc.dram_tensor("WQKV", [128, NC_, 768], BF16, kind="ExternalInput")
    WP = nc.dram_tensor("WP", [128, NH, D], BF16, kind="ExternalInput")
    COS2 = nc.dram_tensor("COS2", [128, NT, HD], BF16, kind="ExternalInput")
    SIN2 = nc.dram_tensor("SIN2", [128, NT, HD], BF16, kind="ExternalInput")
    GSM = nc.dram_tensor("GSM", [1, NH], F32, kind="ExternalInput")
    IDENT = nc.dram_tensor("IDENT", [128, 128], BF16, kind="ExternalInput")
    ONES = nc.dram_tensor("ONES", [128, 1], BF16, kind="ExternalInput")
    TRI = nc.dram_tensor("TRI", [128, 128], BF16, kind="ExternalInput")
    Y = nc.dram_tensor("Y", [S, D], BF16, kind="ExternalOutput")

    with tile.TileContext(nc) as tc:
        with (
            tc.tile_pool(name="const", bufs=1) as const,
            tc.tile_pool(name="w", bufs=8) as wpool,
            tc.tile_pool(name="wp", bufs=1) as wppool,
            tc.tile_pool(name="stream", bufs=4) as stream,
            tc.tile_pool(name="small", bufs=3) as small,
            tc.tile_pool(name="rope", bufs=3) as ropep,
            tc.tile_pool(name="diag", bufs=6) as diagp,
            tc.tile_pool(name="big", bufs=1) as big,
            tc.tile_pool(name="yt", bufs=2) as ytp,
            tc.tile_pool(name="probs", bufs=6) as probsp,
            tc.tile_pool(name="norm", bufs=2) as normp,
            tc.tile_pool(name="outsb", bufs=4) as outsb,
            tc.tile_pool(name="psBig", bufs=4, space="PSUM") as psBig,   # 4 banks
            tc.tile_pool(name="psO", bufs=2, space="PSUM") as psO,       # 2 banks
            tc.tile_pool(name="psR", bufs=2, space="PSUM") as psR,       # 2 banks
        ):
            ident = const.tile([128, 128], BF16)
            nc.sync.dma_start(ident[:], IDENT[:])
            ones = const.tile([128, 1], BF16)
            nc.sync.dma_start(ones[:], ONES[:])
            tri = const.tile([128, 128], BF16)
            nc.sync.dma_start(tri[:], TRI[:])
            gsm = const.tile([1, NH], F32)
            nc.sync.dma_start(gsm[:], GSM[:])
            gsm_bc = const.tile([128, NH], F32)
            nc.gpsimd.partition_broadcast(gsm_bc[:], gsm[:])

            wqkv = []
            for c2 in range(8):
                wt = wpool.tile([128, 2, 768], BF16, tag="w")
                nc.scalar.dma_start(wt[:], WQKV[:, 2 * c2:2 * c2 + 2, :])
                wqkv.append(wt)

            cos2 = const.tile([128, NT, HD], BF16)
            nc.gpsimd.dma_start(cos2[:], COS2[:])
            sin2 = const.tile([128, NT, HD], BF16)
            nc.gpsimd.dma_start(sin2[:], SIN2[:])

            wp = wppool.tile([128, NH, D], BF16)
            nc.scalar.dma_start(wp[:], WP[:])

            qT = big.tile([128, NH, S], BF16)
            kT = big.tile([128, S], BF16)
            v_nat = big.tile([128, NT, HD], BF16)
            rnk_all = big.tile([128, NT], F32)

            pending = [None]

            def flush():
                if pending[0] is not None:
                    pending[0]()
                    pending[0] = None

            yt_prev = [None]
            dmaq = [0]
            proj_queue = []

            def one_proj_group(yt_q, si, sl, dq):
                pj = psBig.tile([128, 512], F32, tag="bank",
                                name=f"pj_{si}_{dq}")
                for hi, h in enumerate(HORD):
                    nc.tensor.matmul(
                        pj[:], yt_q[:, h, sl * 128:(sl + 1) * 128],
                        wp[:, h, dq * 512:(dq + 1) * 512],
                        start=(hi == 0), stop=(hi == NH - 1),
                        skip_group_check=True)
                ev = outsb.tile([128, 512], BF16, tag="ev")
                if dq % 2 == 0:
                    nc.scalar.copy(ev[:], pj[:])
                else:
                    nc.vector.tensor_copy(ev[:], pj[:])
                eng = (nc.scalar, nc.sync)[dmaq[0] % 2]
                dmaq[0] += 1
                eng.dma_start(
                    Y[si * 128:(si + 1) * 128,
                      dq * 512:(dq + 1) * 512], ev[:])

            def queue_proj(yt_q, t_src):
                for si in range(4 * t_src, 4 * t_src + 4):
                    sl = si - 4 * t_src
                    for dq in range(4):
                        proj_queue.append((yt_q, si, sl, dq))

            def emit_proj_groups(n):
                for _ in range(min(n, len(proj_queue))):
                    one_proj_group(*proj_queue.pop(0))

            for t in range(T):
                # ---------- QKV for s-tiles 4t .. 4t+3 ----------
                for si in range(4 * t, 4 * t + 4):
                    xs = stream.tile([128, NC_, 128], BF16, tag="xs")
                    nc.sync.dma_start(xs[:, 0:8, :], XTS[:, si, 0:8, :])
                    nc.sync.dma_start(xs[:, 8:16, :], XTS[:, si, 8:16, :])
                    q_ps = psBig.tile([128, 512], F32, tag="bank")
                    kv_ps = psBig.tile([128, 256], F32, tag="bank")
                    for ci in range(NC_):
                        nc.tensor.matmul(q_ps[:], xs[:, ci, :],
                                         wqkv[ci // 2][:, ci % 2, 0:512],
                                         start=(ci == 0), stop=(ci == NC_ - 1))
                        nc.tensor.matmul(kv_ps[:], xs[:, ci, :],
                                         wqkv[ci // 2][:, ci % 2, 512:768],
                                         start=(ci == 0), stop=(ci == NC_ - 1))
                    emit_proj_groups(4)

                    q4 = q_ps.rearrange("p (h d) -> p h d", h=NH)

                    # q sumsq: ACT Square (no accum) + one DVE reduce; k: ACT accum
                    ssq4 = small.tile([128, 8], F32, tag="ssq4")
                    scr_sq = small.tile([128, 512], F32, tag="scrsq")
                    nc.scalar.activation(scr_sq[:], q_ps[:], AF.Square)
                    nc.vector.tensor_reduce(
                        ssq4[:, 0:4], scr_sq.rearrange("p (h d) -> p h d", h=NH),
                        mybir.AxisListType.X, AluOpType.add)
                    scr = small.tile([128, 128], F32, tag="scr")
                    nc.scalar.activation(scr[:], kv_ps[:, 0:128], AF.Square,
                                         accum_out=ssq4[:, 4:5])
                    mn = small.tile([128, 8], F32, tag="mn")
                    nc.vector.tensor_scalar(mn[:, 0:5], ssq4[:, 0:5], 1.0 / HD, EPS,
                                            AluOpType.mult, AluOpType.add)
                    rt = small.tile([128, 8], F32, tag="rt")
                    nc.scalar.activation(rt[:, 0:5], mn[:, 0:5], AF.Sqrt)
                    rn = small.tile([128, 8], F32, tag="rn")
                    nc.vector.reciprocal_approx_fast(out=rn[:, 0:5], in_=rt[:, 0:5])
                    qsc = small.tile([128, 4], F32, tag="qsc")
                    nc.vector.tensor_tensor(qsc[:], rn[:, 0:4], gsm_bc[:],
                                            AluOpType.mult)
                    nc.vector.tensor_copy(rnk_all[:, si:si + 1], rn[:, 4:5])

                    # rope (merged over 4 q heads, unscaled; scale via diag)
                    tcs = ropep.tile([128, 512], BF16, tag="tcs")
                    tsn = ropep.tile([128, 512], BF16, tag="tsn")
                    t4c = tcs.rearrange("p (h d) -> p h d", h=NH)
                    t4s = tsn.rearrange("p (h d) -> p h d", h=NH)
                    cosb = cos2[:, si:si + 1, :].broadcast_to([128, NH, HD])
                    sina = sin2[:, si:si + 1, 0:64].broadcast_to([128, NH, 64])
                    sinb = sin2[:, si:si + 1, 64:128].broadcast_to([128, NH, 64])
                    nc.vector.tensor_tensor(t4c, q4, cosb, AluOpType.mult)
                    nc.vector.tensor_tensor(t4s[:, :, 0:64], q4[:, :, 64:128],
                                            sina, AluOpType.mult)
                    nc.vector.tensor_tensor(t4s[:, :, 64:128], q4[:, :, 0:64],
                                            sinb, AluOpType.mult)
                    # k rope on gpsimd (gpsimd can't read psum: stage via ACT)
                    knat = ropep.tile([128, 128], BF16, tag="knat")
                    nc.scalar.copy(knat[:], kv_ps[:, 0:128])
                    kcs = ropep.tile([128, 128], BF16, tag="kcs")
                    ksn = ropep.tile([128, 128], BF16, tag="ksn")
                    nc.gpsimd.tensor_tensor(kcs[:], knat[:],
                                            cos2[:, si, :], AluOpType.mult)
                    nc.gpsimd.tensor_tensor(ksn[:, 0:64], knat[:, 64:128],
                                            sin2[:, si, 0:64], AluOpType.mult)
                    nc.gpsimd.tensor_tensor(ksn[:, 64:128], knat[:, 0:64],
                                            sin2[:, si, 64:128], AluOpType.mult)
                    knat2 = ropep.tile([128, 128], BF16, tag="knat2")
                    nc.gpsimd.tensor_tensor(knat2[:], kcs[:], ksn[:], AluOpType.add)
                    qnat = ropep.tile([128, 512], BF16, tag="qnat")
                    nc.vector.tensor_tensor(qnat[:], tcs[:], tsn[:], AluOpType.add)

                    # diag(qsc_h) built on gpsimd; transpose-with-scale on PE
                    for h in HORD:
                        dg = diagp.tile([128, 128], BF16, tag="dg")
                        nc.vector.tensor_scalar(
                            dg[:], ident[:], qsc[:, h:h + 1], None,
                            AluOpType.mult)
                        tp = psR.tile([128, 128], F32, tag="r")
                        nc.tensor.matmul(tp[:], qnat[:, h * 128:(h + 1) * 128],
                                         dg[:], start=True, stop=True)
                        if h in (2, 3):
                            nc.scalar.copy(qT[:, h, si * 128:(si + 1) * 128], tp[:])
                        else:
                            nc.vector.tensor_copy(qT[:, h, si * 128:(si + 1) * 128], tp[:])
                    tpk = psR.tile([128, 128], F32, tag="r")
                    nc.tensor.matmul(tpk[:], knat2[:], ident[:], start=True, stop=True)
                    nc.scalar.copy(kT[:, si * 128:(si + 1) * 128], tpk[:])
                    nc.scalar.copy(v_nat[:, si, :], kv_ps[:, 128:256])

                # ---------- attention for q-slice t ----------
                yt_t = ytp.tile([128, NH, 512], BF16, tag="yt")
                nblk = 4 * t + 4
                for hp in (2, 0):
                    o_ps = {}
                    for h in (hp, hp + 1):
                        o_ps[h] = psO.tile([128, 512], F32, tag="o", name=f"o_{t}_{h}")
                    pacc = ytp.tile([128, 1024], BF16, tag="pacc",
                                    name=f"pacc_{t}_{hp}")
                    for j in range(nblk):
                        off = j - 4 * t
                        lo = max(off, 0) * 128
                        pb = probsp.tile([128, 1024], BF16, tag="probs",
                                         name=f"prb_{t}_{hp}_{j}")
                        for u, h in enumerate((hp, hp + 1)):
                            sc = psBig.tile([128, 512], F32, tag="bank",
                                            name=f"sc_{t}_{h}_{j}")
                            nc.tensor.matmul(
                                sc[:, lo:512],
                                kT[:, j * 128:(j + 1) * 128],
                                qT[:, h, t * 512 + lo:(t + 1) * 512],
                                start=True, stop=True)
                            nc.scalar.activation(pb[:, u * 512 + lo:u * 512 + 512],
                                                 sc[:, lo:512],
                                                 AF.Exp, scale=rnk_all[:, j:j + 1])
                            if off >= 0:
                                nc.vector.tensor_tensor(
                                    pb[:, u * 512 + lo:u * 512 + lo + 128],
                                    pb[:, u * 512 + lo:u * 512 + lo + 128],
                                    tri[:], AluOpType.mult)
                        for u, h in enumerate((hp, hp + 1)):
                            nc.tensor.matmul(
                                o_ps[h][:, lo:512], v_nat[:, j, :],
                                pb[:, u * 512 + lo:u * 512 + 512],
                                start=(j == 0), stop=(j == nblk - 1),
                                skip_group_check=True)
                        # probs-sum accumulation off the PE
                        if j == 0:
                            nc.vector.tensor_copy(pacc[:], pb[:])
                        elif j % 2 == 1:
                            nc.vector.tensor_tensor(
                                pacc[:, lo:512], pacc[:, lo:512],
                                pb[:, lo:512], AluOpType.add)
                            nc.vector.tensor_tensor(
                                pacc[:, 512 + lo:1024], pacc[:, 512 + lo:1024],
                                pb[:, 512 + lo:1024], AluOpType.add)
                        else:
                            nc.gpsimd.tensor_tensor(
                                pacc[:, lo:512], pacc[:, lo:512],
                                pb[:, lo:512], AluOpType.add)
                            nc.gpsimd.tensor_tensor(
                                pacc[:, 512 + lo:1024], pacc[:, 512 + lo:1024],
                                pb[:, 512 + lo:1024], AluOpType.add)
                        if j == 0:
                            flush()

                    def make_ep(o_ps=o_ps, pacc=pacc, yt_t=yt_t, hp=hp):
                        def ep():
                            ds = {}
                            for u, h in enumerate((hp, hp + 1)):
                                ds[h] = psR.tile([1, 512], F32, tag="r",
                                                 name=f"ds_{t}_{h}")
                                nc.tensor.matmul(
                                    ds[h][:], ones[:],
                                    pacc[:, u * 512:(u + 1) * 512],
                                    start=True, stop=True)
                            for h in (hp, hp + 1):
                                nc.scalar.copy(yt_t[:, h, :], o_ps[h][:])
                                rr = normp.tile([1, 512], F32, tag="rr")
                                nc.vector.reciprocal_approx_fast(
                                    out=rr[:], in_=ds[h][:])
                                rbc = normp.tile([128, 512], F32, tag="rbc")
                                nc.gpsimd.partition_broadcast(rbc[:], rr[:])
                                nc.vector.tensor_tensor(
                                    yt_t[:, h, :], yt_t[:, h, :], rbc[:],
                                    AluOpType.mult)
                        return ep
                    pending[0] = make_ep()

                # ---------- queue projection of this q-slice ----------
                emit_proj_groups(len(proj_queue))
                flush()
                queue_proj(yt_t, t)
                yt_prev[0] = yt_t

            emit_proj_groups(len(proj_queue))

    nc.compile()
    return nc


def _host_inputs(x, Wq, Wk, Wv, Wproj, q_gain):
    x = np.asarray(x, dtype=np.float32)
    Wq = np.asarray(Wq, dtype=np.float32)
    Wk = np.asarray(Wk, dtype=np.float32)
    Wv = np.asarray(Wv, dtype=np.float32)
    Wproj = np.asarray(Wproj, dtype=np.float32)
    q_gain = np.asarray(q_gain, dtype=np.float32)

    inv = (1.0 / ROPE_BASE ** (np.arange(0, HD, 2, dtype=np.float32) / HD)).astype(np.float32)
    ang = np.outer(np.arange(S, dtype=np.float32), inv)
    cos = np.cos(ang).astype(np.float32)
    sin = np.sin(ang).astype(np.float32)
    cos2 = np.concatenate([cos, cos], 1).reshape(NT, 128, HD).transpose(1, 0, 2)
    sin2 = np.concatenate([sin, -sin], 1).reshape(NT, 128, HD).transpose(1, 0, 2)

    kk = np.arange(128)[:, None]
    qq = np.arange(128)[None, :]
    tri = (kk <= qq).astype(BF)
    ident = np.eye(128, dtype=BF)
    ones = np.ones((128, 1), dtype=BF)

    in_maps = []
    for cid in range(8):
        b, g = cid // 4, cid % 4
        # x[b].T [D, S] -> [128, NT, NC_, 128]: (c%128, s//128, c//128, s%128)
        xts = np.ascontiguousarray(
            x[b].T.reshape(NC_, 128, NT, 128).transpose(1, 2, 0, 3)).astype(BF)
        wq = Wq[g * 512:(g + 1) * 512, :].T            # [D, 512]
        wk = Wk[g * 128:(g + 1) * 128, :].T            # [D, 128]
        wv = Wv[g * 128:(g + 1) * 128, :].T
        wqkv = np.concatenate([wq, wk, wv], 1)         # [D, 768]
        wpp = Wproj[:, g * 512:(g + 1) * 512].T        # [512, D]
        in_maps.append({
            "XTS": xts,
            "WQKV": np.ascontiguousarray(
                wqkv.reshape(NC_, 128, 768).transpose(1, 0, 2)).astype(BF),
            "WP": np.ascontiguousarray(
                wpp.reshape(NH, 128, D).transpose(1, 0, 2)).astype(BF),
            "COS2": np.ascontiguousarray(cos2).astype(BF),
            "SIN2": np.ascontiguousarray(sin2).astype(BF),
            "GSM": (q_gain[g * 4:(g + 1) * 4] * SM).reshape(1, NH).astype(np.float32),
            "IDENT": ident, "ONES": ones, "TRI": tri,
        })
    return in_maps


def _get_prog():
    global _PROG
    if _PROG is None:
        _PROG = _build_program()
    return _PROG


def kernel(x, Wq, Wk, Wv, Wproj, q_gain, _trace=False, _tmpdir=None):
    from concourse.bass_utils import run_bass_kernel_spmd
    nc = _get_prog()
    in_maps = _host_inputs(x, Wq, Wk, Wv, Wproj, q_gain)
    kwargs = {}
    if _tmpdir is not None:
        os.makedirs(_tmpdir, exist_ok=True)
        kwargs["tmpdir"] = _tmpdir
    res = run_bass_kernel_spmd(nc, in_maps, list(range(8)), trace=_trace, **kwargs)
    y = np.empty((B, S, D), dtype=np.float32)
    for b in range(B):
        acc = res.results[4 * b]["Y"].astype(np.float32)
        for g in range(1, 4):
            acc = acc + res.results[4 * b + g]["Y"].astype(np.float32)
        y[b] = acc
    if _trace:
        kernel._last_result = res
    return y


# revision 20
# speedup vs baseline: 1.0013x; 1.0013x over previous
"""Causal GQA self-attention (B=2, S=2048, D=2048, H=16, KV=4) on 8 TRN2 cores.

Sharding: core = (b, g) with b = batch (2) x g = kv-head group (4).
Each core computes 4 q-heads / 1 kv-head for one batch and a partial
projection output [S, D]; host sums the 4 group partials per batch.

v3: all-bf16 matmuls, per-t merged loop with proj of t-1 interleaved
after attention t, host-preswizzled x (contiguous DMA), q-norm via DVE
bn_stats, q norm-scale folded into the transpose as a diag matmul,
k norm-scale folded into exp's per-partition scale, rope add folded
into psum-accumulated transpose pair, proj written psum->DRAM directly.
"""
import os
import sys

if '/opt/trn_rl_repo' not in sys.path:
    sys.path.insert(0, '/opt/trn_rl_repo')

import numpy as np
import ml_dtypes

BF = ml_dtypes.bfloat16

B, S, D = 2, 2048, 2048
NH_TOT, NKV_TOT, HD = 16, 4, 128
NH = 4                 # q heads per core
NT = S // 128          # 16 s-tiles
NC_ = D // 128         # 16 c-tiles
T = 4                  # q-slices of 512
SM = 1.0 / np.sqrt(HD)
EPS = float(np.finfo(np.float32).eps)
ROPE_BASE = 10000.0
HORD = (2, 3, 0, 1)    # head emission order (attention runs pair hp=2 first)

_PROG = None


def _build_program():
    import concourse.bass as bass
    import concourse.mybir as mybir
    import concourse.tile as tile
    from concourse import bacc
    from concourse.alu_op_type import AluOpType

    F32 = mybir.dt.float32
    BF16 = mybir.dt.bfloat16
    AF = mybir.ActivationFunctionType

    nc = bacc.Bacc("TRN2", target_bir_lowering=False, debug=False)

    XTS = nc.dram_tensor("XTS", [128, NT, NC_, 128], BF16, kind="ExternalInput")
    WQKV = nc.dram_tensor("WQKV", [128, NC_, 768], BF16, kind="ExternalInput")
    WP = nc.dram_tensor("WP", [128, NH, D], BF16, kind="ExternalInput")
    COS2 = nc.dram_tensor("COS2", [128, NT, HD], BF16, kind="ExternalInput")
    SIN2 = nc.dram_tensor("SIN2", [128, NT, HD], BF16, kind="ExternalInput")
    GSM = nc.dram_tensor("GSM", [1, NH], F32, kind="ExternalInput")
    IDENT = nc.dram_tensor("IDENT", [128, 128], BF16, kind="ExternalInput")
    ONES = nc.dram_tensor("ONES", [128, 1], BF16, kind="ExternalInput")
    TRI = nc.dram_tensor("TRI", [128, 128], BF16, kind="ExternalInput")
    Y = nc.dram_tensor("Y", [S, D], BF16, kind="ExternalOutput")

    with tile.TileContext(nc) as tc:
        with (
            tc.tile_pool(name="const", bufs=1) as const,
            tc.tile_pool(name="w", bufs=16) as wpool,
            tc.tile_pool(name="wp", bufs=1) as wppool,
            tc.tile_pool(name="stream", bufs=4) as stream,
            tc.tile_pool(name="small", bufs=3) as small,
            tc.tile_pool(name="rope", bufs=3) as ropep,
            tc.tile_pool(name="diag", bufs=6) as diagp,
            tc.tile_pool(name="big", bufs=1) as big,
            tc.tile_pool(name="yt", bufs=2) as ytp,
            tc.tile_pool(name="probs", bufs=6) as probsp,
            tc.tile_pool(name="norm", bufs=2) as normp,
            tc.tile_pool(name="outsb", bufs=4) as outsb,
            tc.tile_pool(name="psBig", bufs=4, space="PSUM") as psBig,   # 4 banks
            tc.tile_pool(name="psO", bufs=2, space="PSUM") as psO,       # 2 banks
            tc.tile_pool(name="psR", bufs=2, space="PSUM") as psR,       # 2 banks
        ):
            ident = const.tile([128, 128], BF16)
            nc.sync.dma_start(ident[:], IDENT[:])
            ones = const.tile([128, 1], BF16)
            nc.sync.dma_start(ones[:], ONES[:])
            tri = const.tile([128, 128], BF16)
            nc.sync.dma_start(tri[:], TRI[:])
            gsm = const.tile([1, NH], F32)
            nc.sync.dma_start(gsm[:], GSM[:])
            gsm_bc = const.tile([128, NH], F32)
            nc.gpsimd.partition_broadcast(gsm_bc[:], gsm[:])

            wqkv = []
            for c1 in range(NC_):
                wt = wpool.tile([128, 1, 768], BF16, tag="w")
                nc.scalar.dma_start(wt[:], WQKV[:, c1:c1 + 1, :])
                wqkv.append(wt)

            cos2 = const.tile([128, NT, HD], BF16)
            nc.gpsimd.dma_start(cos2[:], COS2[:])
            sin2 = const.tile([128, NT, HD], BF16)
            nc.gpsimd.dma_start(sin2[:], SIN2[:])

            wp = wppool.tile([128, NH, D], BF16)
            nc.scalar.dma_start(wp[:], WP[:])

            qT = big.tile([128, NH, S], BF16)
            kT = big.tile([128, S], BF16)
            v_nat = big.tile([128, NT, HD], BF16)
            rnk_all = big.tile([128, NT], F32)

            pending = [None]

            def flush():
                if pending[0] is not None:
                    pending[0]()
                    pending[0] = None

            yt_prev = [None]
            dmaq = [0]
            proj_queue = []

            def one_proj_group(yt_q, si, sl, dq):
                pj = psBig.tile([128, 512], F32, tag="bank",
                                name=f"pj_{si}_{dq}")
                for hi, h in enumerate(HORD):
                    nc.tensor.matmul(
                        pj[:], yt_q[:, h, sl * 128:(sl + 1) * 128],
                        wp[:, h, dq * 512:(dq + 1) * 512],
                        start=(hi == 0), stop=(hi == NH - 1),
                        skip_group_check=True)
                ev = outsb.tile([128, 512], BF16, tag="ev")
                if dq % 2 == 0:
                    nc.scalar.copy(ev[:], pj[:])
                else:
                    nc.vector.tensor_copy(ev[:], pj[:])
                eng = (nc.scalar, nc.sync)[dmaq[0] % 2]
                dmaq[0] += 1
                eng.dma_start(
                    Y[si * 128:(si + 1) * 128,
                      dq * 512:(dq + 1) * 512], ev[:])

            def queue_proj(yt_q, t_src):
                for si in range(4 * t_src, 4 * t_src + 4):
                    sl = si - 4 * t_src
                    for dq in range(4):
                        proj_queue.append((yt_q, si, sl, dq))

            def emit_proj_groups(n):
                for _ in range(min(n, len(proj_queue))):
                    one_proj_group(*proj_queue.pop(0))

            for t in range(T):
                # ---------- QKV for s-tiles 4t .. 4t+3 ----------
                for si in range(4 * t, 4 * t + 4):
                    xs = stream.tile([128, NC_, 128], BF16, tag="xs")
                    if si == 0:
                        for q_ in range(4):
                            nc.sync.dma_start(xs[:, 4 * q_:4 * q_ + 4, :],
                                              XTS[:, si, 4 * q_:4 * q_ + 4, :])
                    else:
                        nc.sync.dma_start(xs[:, 0:8, :], XTS[:, si, 0:8, :])
                        nc.sync.dma_start(xs[:, 8:16, :], XTS[:, si, 8:16, :])
                    q_ps = psBig.tile([128, 512], F32, tag="bank")
                    kv_ps = psBig.tile([128, 256], F32, tag="bank")
                    for ci in range(NC_):
                        nc.tensor.matmul(q_ps[:], xs[:, ci, :],
                                         wqkv[ci][:, 0, 0:512],
                                         start=(ci == 0), stop=(ci == NC_ - 1))
                        nc.tensor.matmul(kv_ps[:], xs[:, ci, :],
                                         wqkv[ci][:, 0, 512:768],
                                         start=(ci == 0), stop=(ci == NC_ - 1))
                    emit_proj_groups(4)

                    q4 = q_ps.rearrange("p (h d) -> p h d", h=NH)

                    # q sumsq: ACT Square (no accum) + one DVE reduce; k: ACT accum
                    ssq4 = small.tile([128, 8], F32, tag="ssq4")
                    scr_sq = small.tile([128, 512], F32, tag="scrsq")
                    nc.scalar.activation(scr_sq[:], q_ps[:], AF.Square)
                    nc.vector.tensor_reduce(
                        ssq4[:, 0:4], scr_sq.rearrange("p (h d) -> p h d", h=NH),
                        mybir.AxisListType.X, AluOpType.add)
                    scr = small.tile([128, 128], F32, tag="scr")
                    nc.scalar.activation(scr[:], kv_ps[:, 0:128], AF.Square,
                                         accum_out=ssq4[:, 4:5])
                    mn = small.tile([128, 8], F32, tag="mn")
                    nc.vector.tensor_scalar(mn[:, 0:5], ssq4[:, 0:5], 1.0 / HD, EPS,
                                            AluOpType.mult, AluOpType.add)
                    rt = small.tile([128, 8], F32, tag="rt")
                    nc.scalar.activation(rt[:, 0:5], mn[:, 0:5], AF.Sqrt)
                    rn = small.tile([128, 8], F32, tag="rn")
                    nc.vector.reciprocal_approx_fast(out=rn[:, 0:5], in_=rt[:, 0:5])
                    qsc = small.tile([128, 4], F32, tag="qsc")
                    nc.vector.tensor_tensor(qsc[:], rn[:, 0:4], gsm_bc[:],
                                            AluOpType.mult)
                    nc.vector.tensor_copy(rnk_all[:, si:si + 1], rn[:, 4:5])

                    # rope (merged over 4 q heads, unscaled; scale via diag)
                    tcs = ropep.tile([128, 512], BF16, tag="tcs")
                    tsn = ropep.tile([128, 512], BF16, tag="tsn")
                    t4c = tcs.rearrange("p (h d) -> p h d", h=NH)
                    t4s = tsn.rearrange("p (h d) -> p h d", h=NH)
                    cosb = cos2[:, si:si + 1, :].broadcast_to([128, NH, HD])
                    sina = sin2[:, si:si + 1, 0:64].broadcast_to([128, NH, 64])
                    sinb = sin2[:, si:si + 1, 64:128].broadcast_to([128, NH, 64])
                    nc.vector.tensor_tensor(t4c, q4, cosb, AluOpType.mult)
                    nc.vector.tensor_tensor(t4s[:, :, 0:64], q4[:, :, 64:128],
                                            sina, AluOpType.mult)
                    nc.vector.tensor_tensor(t4s[:, :, 64:128], q4[:, :, 0:64],
                                            sinb, AluOpType.mult)
                    # k rope on gpsimd (gpsimd can't read psum: stage via ACT)
                    knat = ropep.tile([128, 128], BF16, tag="knat")
                    nc.scalar.copy(knat[:], kv_ps[:, 0:128])
                    kcs = ropep.tile([128, 128], BF16, tag="kcs")
                    ksn = ropep.tile([128, 128], BF16, tag="ksn")
                    nc.gpsimd.tensor_tensor(kcs[:], knat[:],
                                            cos2[:, si, :], AluOpType.mult)
                    nc.gpsimd.tensor_tensor(ksn[:, 0:64], knat[:, 64:128],
                                            sin2[:, si, 0:64], AluOpType.mult)
                    nc.gpsimd.tensor_tensor(ksn[:, 64:128], knat[:, 0:64],
                                            sin2[:, si, 64:128], AluOpType.mult)
                    knat2 = ropep.tile([128, 128], BF16, tag="knat2")
                    nc.gpsimd.tensor_tensor(knat2[:], kcs[:], ksn[:], AluOpType.add)
                    qnat = ropep.tile([128, 512], BF16, tag="qnat")
                    nc.vector.tensor_tensor(qnat[:], tcs[:], tsn[:], AluOpType.add)

                    # diag(qsc_h) built on gpsimd; transpose-with-scale on PE
                    for h in HORD:
                        dg = diagp.tile([128, 128], BF16, tag="dg")
                        nc.vector.tensor_scalar(
                            dg[:], ident[:], qsc[:, h:h + 1], None,
                            AluOpType.mult)
                        tp = psR.tile([128, 128], F32, tag="r")
                        nc.tensor.matmul(tp[:], qnat[:, h * 128:(h + 1) * 128],
                                         dg[:], start=True, stop=True)
                        if h in (2, 3):
                            nc.scalar.copy(qT[:, h, si * 128:(si + 1) * 128], tp[:])
                        else:
                            nc.vector.tensor_copy(qT[:, h, si * 128:(si + 1) * 128], tp[:])
                    tpk = psR.tile([128, 128], F32, tag="r")
                    nc.tensor.matmul(tpk[:], knat2[:], ident[:], start=True, stop=True)
                    nc.scalar.copy(kT[:, si * 128:(si + 1) * 128], tpk[:])
                    nc.scalar.copy(v_nat[:, si, :], kv_ps[:, 128:256])

                # ---------- attention for q-slice t ----------
                yt_t = ytp.tile([128, NH, 512], BF16, tag="yt")
                nblk = 4 * t + 4
                for hp in (2, 0):
                    o_ps = {}
                    rs_ps = {}
                    for h in (hp, hp + 1):
                        o_ps[h] = psO.tile([128, 512], F32, tag="o", name=f"o_{t}_{h}")
                        rs_ps[h] = psR.tile([1, 512], F32, tag="r", name=f"rs_{t}_{h}")
                    for j in range(nblk):
                        off = j - 4 * t
                        lo = max(off, 0) * 128
                        prb = {}
                        for h in (hp, hp + 1):
                            sc = psBig.tile([128, 512], F32, tag="bank",
                                            name=f"sc_{t}_{h}_{j}")
                            nc.tensor.matmul(
                                sc[:, lo:512],
                                kT[:, j * 128:(j + 1) * 128],
                                qT[:, h, t * 512 + lo:(t + 1) * 512],
                                start=True, stop=True)
                            pb = probsp.tile([128, 512], BF16, tag="probs",
                                             name=f"prb_{t}_{h}_{j}")
                            nc.scalar.activation(pb[:, lo:512], sc[:, lo:512],
                                                 AF.Exp, scale=rnk_all[:, j:j + 1])
                            if off >= 0:
                                nc.vector.tensor_tensor(
                                    pb[:, lo:lo + 128], pb[:, lo:lo + 128],
                                    tri[:], AluOpType.mult)
                            prb[h] = pb
                        for h in (hp, hp + 1):
                            nc.tensor.matmul(
                                o_ps[h][:, lo:512], v_nat[:, j, :], prb[h][:, lo:512],
                                start=(j == 0), stop=(j == nblk - 1),
                                skip_group_check=True)
                            nc.tensor.matmul(
                                rs_ps[h][:, lo:512], ones[:], prb[h][:, lo:512],
                                start=(j == 0), stop=(j == nblk - 1),
                                skip_group_check=True)
                        if j == 0:
                            flush()

                    def make_ep(o_ps=o_ps, rs_ps=rs_ps, yt_t=yt_t, hp=hp):
                        def ep():
                            for h in (hp, hp + 1):
                                nc.scalar.copy(yt_t[:, h, :], o_ps[h][:])
                                rr = normp.tile([1, 512], F32, tag="rr")
                                nc.vector.reciprocal_approx_fast(
                                    out=rr[:], in_=rs_ps[h][:])
                                rbc = normp.tile([128, 512], F32, tag="rbc")
                                nc.gpsimd.partition_broadcast(rbc[:], rr[:])
                                nc.vector.tensor_tensor(
                                    yt_t[:, h, :], yt_t[:, h, :], rbc[:],
                                    AluOpType.mult)
                        return ep
                    pending[0] = make_ep()

                # ---------- queue projection of this q-slice ----------
                emit_proj_groups(len(proj_queue))
                flush()
                queue_proj(yt_t, t)
                yt_prev[0] = yt_t

            emit_proj_groups(len(proj_queue))

    nc.compile()
    return nc


def _host_inputs(x, Wq, Wk, Wv, Wproj, q_gain):
    x = np.asarray(x, dtype=np.float32)
    Wq = np.asarray(Wq, dtype=np.float32)
    Wk = np.asarray(Wk, dtype=np.float32)
    Wv = np.asarray(Wv, dtype=np.float32)
    Wproj = np.asarray(Wproj, dtype=np.float32)
    q_gain = np.asarray(q_gain, dtype=np.float32)

    inv = (1.0 / ROPE_BASE ** (np.arange(0, HD, 2, dtype=np.float32) / HD)).astype(np.float32)
    ang = np.outer(np.arange(S, dtype=np.float32), inv)
    cos = np.cos(ang).astype(np.float32)
    sin = np.sin(ang).astype(np.float32)
    cos2 = np.concatenate([cos, cos], 1).reshape(NT, 128, HD).transpose(1, 0, 2)
    sin2 = np.concatenate([sin, -sin], 1).reshape(NT, 128, HD).transpose(1, 0, 2)

    kk = np.arange(128)[:, None]
    qq = np.arange(128)[None, :]
    tri = (kk <= qq).astype(BF)
    ident = np.eye(128, dtype=BF)
    ones = np.ones((128, 1), dtype=BF)

    in_maps = []
    for cid in range(8):
        b, g = cid // 4, cid % 4
        # x[b].T [D, S] -> [128, NT, NC_, 128]: (c%128, s//128, c//128, s%128)
        xts = np.ascontiguousarray(
            x[b].T.reshape(NC_, 128, NT, 128).transpose(1, 2, 0, 3)).astype(BF)
        wq = Wq[g * 512:(g + 1) * 512, :].T            # [D, 512]
        wk = Wk[g * 128:(g + 1) * 128, :].T            # [D, 128]
        wv = Wv[g * 128:(g + 1) * 128, :].T
        wqkv = np.concatenate([wq, wk, wv], 1)         # [D, 768]
        wpp = Wproj[:, g * 512:(g + 1) * 512].T        # [512, D]
        in_maps.append({
            "XTS": xts,
            "WQKV": np.ascontiguousarray(
                wqkv.reshape(NC_, 128, 768).transpose(1, 0, 2)).astype(BF),
            "WP": np.ascontiguousarray(
                wpp.reshape(NH, 128, D).transpose(1, 0, 2)).astype(BF),
            "COS2": np.ascontiguousarray(cos2).astype(BF),
            "SIN2": np.ascontiguousarray(sin2).astype(BF),
            "GSM": (q_gain[g * 4:(g + 1) * 4] * SM).reshape(1, NH).astype(np.float32),
            "IDENT": ident, "ONES": ones, "TRI": tri,
        })
    return in_maps


def _get_prog():
    global _PROG
    if _PROG is None:
        _PROG = _build_program()
    return _PROG


def kernel(x, Wq, Wk, Wv, Wproj, q_gain, _trace=False, _tmpdir=None):
    from concourse.bass_utils import run_bass_kernel_spmd
    nc = _get_prog()
    in_maps = _host_inputs(x, Wq, Wk, Wv, Wproj, q_gain)
    kwargs = {}
    if _tmpdir is not None:
        os.makedirs(_tmpdir, exist_ok=True)
        kwargs["tmpdir"] = _tmpdir
    res = run_bass_kernel_spmd(nc, in_maps, list(range(8)), trace=_trace, **kwargs)
    y = np.empty((B, S, D), dtype=np.float32)
    for b in range(B):
        acc = res.results[4 * b]["Y"].astype(np.float32)
        for g in range(1, 4):
            acc = acc + res.results[4 * b + g]["Y"].astype(np.float32)
        y[b] = acc
    if _trace:
        kernel._last_result = res
    return y


# revision 21
# speedup vs baseline: 1.1901x; 1.1885x over previous
"""Causal GQA self-attention (B=2, S=2048, D=2048, H=16, KV=4) on 8 TRN2 cores.

Sharding: core = (b, g) with b = batch (2) x g = kv-head group (4).
Each core computes 4 q-heads / 1 kv-head for one batch and a partial
projection output [S, D]; host sums the 4 group partials per batch.

v3: all-bf16 matmuls, per-t merged loop with proj of t-1 interleaved
after attention t, host-preswizzled x (contiguous DMA), q-norm via DVE
bn_stats, q norm-scale folded into the transpose as a diag matmul,
k norm-scale folded into exp's per-partition scale, rope add folded
into psum-accumulated transpose pair, proj written psum->DRAM directly.
"""
import os
import sys

if '/opt/trn_rl_repo' not in sys.path:
    sys.path.insert(0, '/opt/trn_rl_repo')

import numpy as np
import ml_dtypes

BF = ml_dtypes.bfloat16

B, S, D = 2, 2048, 2048
NH_TOT, NKV_TOT, HD = 16, 4, 128
NH = 4                 # q heads per core
NT = S // 128          # 16 s-tiles
NC_ = D // 128         # 16 c-tiles
T = 4                  # q-slices of 512
SM = 1.0 / np.sqrt(HD)
EPS = float(np.finfo(np.float32).eps)
ROPE_BASE = 10000.0
HORD = (2, 3, 0, 1)    # head emission order (attention runs pair hp=2 first)

_PROG = None


def _build_program():
    import concourse.bass as bass
    import concourse.mybir as mybir
    import concourse.tile as tile
    from concourse import bacc
    from concourse.alu_op_type import AluOpType

    F32 = mybir.dt.float32
    BF16 = mybir.dt.bfloat16
    AF = mybir.ActivationFunctionType

    nc = bacc.Bacc("TRN2", target_bir_lowering=False, debug=False)

    XTS = nc.dram_tensor("XTS", [128, NT, NC_, 128], BF16, kind="ExternalInput")
    WQKV = nc.dram_tensor("WQKV", [128, NC_, 768], BF16, kind="ExternalInput")
    WP = nc.dram_tensor("WP", [128, NH, D], BF16, kind="ExternalInput")
    COS2 = nc.dram_tensor("COS2", [128, NT, HD], BF16, kind="ExternalInput")
    SIN2 = nc.dram_tensor("SIN2", [128, NT, HD], BF16, kind="ExternalInput")
    GSM = nc.dram_tensor("GSM", [1, NH], F32, kind="ExternalInput")
    IDENT = nc.dram_tensor("IDENT", [128, 128], BF16, kind="ExternalInput")
    ONES = nc.dram_tensor("ONES", [128, 1], BF16, kind="ExternalInput")
    TRI = nc.dram_tensor("TRI", [128, 128], BF16, kind="ExternalInput")
    Y = nc.dram_tensor("Y", [S, D], BF16, kind="ExternalOutput")

    with tile.TileContext(nc) as tc:
        with (
            tc.tile_pool(name="const", bufs=1) as const,
            tc.tile_pool(name="w", bufs=8) as wpool,
            tc.tile_pool(name="wp", bufs=1) as wppool,
            tc.tile_pool(name="stream", bufs=4) as stream,
            tc.tile_pool(name="small", bufs=3) as small,
            tc.tile_pool(name="rope", bufs=3) as ropep,
            tc.tile_pool(name="diag", bufs=6) as diagp,
            tc.tile_pool(name="big", bufs=1) as big,
            tc.tile_pool(name="yt", bufs=2) as ytp,
            tc.tile_pool(name="probs", bufs=6) as probsp,
            tc.tile_pool(name="norm", bufs=2) as normp,
            tc.tile_pool(name="outsb", bufs=4) as outsb,
            tc.tile_pool(name="psBig", bufs=4, space="PSUM") as psBig,   # 4 banks
            tc.tile_pool(name="psO", bufs=2, space="PSUM") as psO,       # 2 banks
            tc.tile_pool(name="psR", bufs=2, space="PSUM") as psR,       # 2 banks
        ):
            ident = const.tile([128, 128], BF16)
            nc.sync.dma_start(ident[:], IDENT[:])
            ones = const.tile([128, 1], BF16)
            nc.sync.dma_start(ones[:], ONES[:])
            tri = const.tile([128, 128], BF16)
            nc.sync.dma_start(tri[:], TRI[:])
            gsm = const.tile([1, NH], F32)
            nc.sync.dma_start(gsm[:], GSM[:])
            gsm_bc = const.tile([128, NH], F32)
            nc.gpsimd.partition_broadcast(gsm_bc[:], gsm[:])

            wqkv = []
            for c2 in range(8):
                wt = wpool.tile([128, 2, 768], BF16, tag="w")
                nc.scalar.dma_start(wt[:], WQKV[:, 2 * c2:2 * c2 + 2, :])
                wqkv.append(wt)

            cos2 = const.tile([128, NT, HD], BF16)
            nc.gpsimd.dma_start(cos2[:], COS2[:])
            sin2 = const.tile([128, NT, HD], BF16)
            nc.gpsimd.dma_start(sin2[:], SIN2[:])

            wp = wppool.tile([128, NH, D], BF16)
            nc.scalar.dma_start(wp[:], WP[:])

            qT = big.tile([128, NH, S], BF16)
            kT = big.tile([128, S], BF16)
            v_nat = big.tile([128, NT, HD], BF16)
            rnk_all = big.tile([128, NT], F32)

            pending = [None]

            def flush():
                if pending[0] is not None:
                    pending[0]()
                    pending[0] = None

            yt_prev = [None]
            dmaq = [0]
            proj_queue = []

            def one_proj_group(yt_q, si, sl, dq):
                pj = psBig.tile([128, 512], F32, tag="bank",
                                name=f"pj_{si}_{dq}")
                for hi, h in enumerate(HORD):
                    nc.tensor.matmul(
                        pj[:], yt_q[:, h, sl * 128:(sl + 1) * 128],
                        wp[:, h, dq * 512:(dq + 1) * 512],
                        start=(hi == 0), stop=(hi == NH - 1),
                        skip_group_check=True)
                ev = outsb.tile([128, 512], BF16, tag="ev")
                if dq % 2 == 0:
                    nc.scalar.copy(ev[:], pj[:])
                else:
                    nc.vector.tensor_copy(ev[:], pj[:])
                eng = (nc.scalar, nc.sync)[dmaq[0] % 2]
                dmaq[0] += 1
                eng.dma_start(
                    Y[si * 128:(si + 1) * 128,
                      dq * 512:(dq + 1) * 512], ev[:])

            def queue_proj(yt_q, t_src):
                for si in range(4 * t_src, 4 * t_src + 4):
                    sl = si - 4 * t_src
                    for dq in range(4):
                        proj_queue.append((yt_q, si, sl, dq))

            def emit_proj_groups(n):
                for _ in range(min(n, len(proj_queue))):
                    one_proj_group(*proj_queue.pop(0))

            for t in range(T):
                # ---------- QKV for s-tiles 4t .. 4t+3 ----------
                for si in range(4 * t, 4 * t + 4):
                    xs = stream.tile([128, NC_, 128], BF16, tag="xs")
                    nc.sync.dma_start(xs[:, 0:8, :], XTS[:, si, 0:8, :])
                    nc.sync.dma_start(xs[:, 8:16, :], XTS[:, si, 8:16, :])
                    q_ps = psBig.tile([128, 512], F32, tag="bank")
                    kv_ps = psBig.tile([128, 256], F32, tag="bank")
                    for ci in range(NC_):
                        nc.tensor.matmul(q_ps[:], xs[:, ci, :],
                                         wqkv[ci // 2][:, ci % 2, 0:512],
                                         start=(ci == 0), stop=(ci == NC_ - 1))
                        nc.tensor.matmul(kv_ps[:], xs[:, ci, :],
                                         wqkv[ci // 2][:, ci % 2, 512:768],
                                         start=(ci == 0), stop=(ci == NC_ - 1))
                    emit_proj_groups(4)

                    q4 = q_ps.rearrange("p (h d) -> p h d", h=NH)

                    # q sumsq: ACT Square (no accum) + one DVE reduce; k: ACT accum
                    ssq4 = small.tile([128, 8], F32, tag="ssq4")
                    scr_sq = small.tile([128, 512], F32, tag="scrsq")
                    nc.scalar.activation(scr_sq[:], q_ps[:], AF.Square)
                    nc.vector.tensor_reduce(
                        ssq4[:, 0:4], scr_sq.rearrange("p (h d) -> p h d", h=NH),
                        mybir.AxisListType.X, AluOpType.add)
                    scr = small.tile([128, 128], F32, tag="scr")
                    nc.scalar.activation(scr[:], kv_ps[:, 0:128], AF.Square,
                                         accum_out=ssq4[:, 4:5])
                    mn = small.tile([128, 8], F32, tag="mn")
                    nc.vector.tensor_scalar(mn[:, 0:5], ssq4[:, 0:5], 1.0 / HD, EPS,
                                            AluOpType.mult, AluOpType.add)
                    rt = small.tile([128, 8], F32, tag="rt")
                    nc.scalar.activation(rt[:, 0:5], mn[:, 0:5], AF.Sqrt)
                    rn = small.tile([128, 8], F32, tag="rn")
                    nc.vector.reciprocal_approx_fast(out=rn[:, 0:5], in_=rt[:, 0:5])
                    qsc = small.tile([128, 4], F32, tag="qsc")
                    nc.vector.tensor_tensor(qsc[:], rn[:, 0:4], gsm_bc[:],
                                            AluOpType.mult)
                    nc.vector.tensor_copy(rnk_all[:, si:si + 1], rn[:, 4:5])

                    # rope (merged over 4 q heads, unscaled; scale via diag)
                    tcs = ropep.tile([128, 512], BF16, tag="tcs")
                    tsn = ropep.tile([128, 512], BF16, tag="tsn")
                    t4c = tcs.rearrange("p (h d) -> p h d", h=NH)
                    t4s = tsn.rearrange("p (h d) -> p h d", h=NH)
                    cosb = cos2[:, si:si + 1, :].broadcast_to([128, NH, HD])
                    sina = sin2[:, si:si + 1, 0:64].broadcast_to([128, NH, 64])
                    sinb = sin2[:, si:si + 1, 64:128].broadcast_to([128, NH, 64])
                    nc.vector.tensor_tensor(t4c, q4, cosb, AluOpType.mult)
                    nc.vector.tensor_tensor(t4s[:, :, 0:64], q4[:, :, 64:128],
                                            sina, AluOpType.mult)
                    nc.vector.tensor_tensor(t4s[:, :, 64:128], q4[:, :, 0:64],
                                            sinb, AluOpType.mult)
                    # k rope on gpsimd (gpsimd can't read psum: stage via ACT)
                    knat = ropep.tile([128, 128], BF16, tag="knat")
                    nc.scalar.copy(knat[:], kv_ps[:, 0:128])
                    kcs = ropep.tile([128, 128], BF16, tag="kcs")
                    ksn = ropep.tile([128, 128], BF16, tag="ksn")
                    nc.gpsimd.tensor_tensor(kcs[:], knat[:],
                                            cos2[:, si, :], AluOpType.mult)
                    nc.gpsimd.tensor_tensor(ksn[:, 0:64], knat[:, 64:128],
                                            sin2[:, si, 0:64], AluOpType.mult)
                    nc.gpsimd.tensor_tensor(ksn[:, 64:128], knat[:, 0:64],
                                            sin2[:, si, 64:128], AluOpType.mult)

                    # diag(qsc_h) built on gpsimd; transpose-with-scale on PE
                    for h in HORD:
                        dg = diagp.tile([128, 128], BF16, tag="dg")
                        nc.vector.tensor_scalar(
                            dg[:], ident[:], qsc[:, h:h + 1], None,
                            AluOpType.mult)
                        tp = psR.tile([128, 128], F32, tag="r")
                        nc.tensor.matmul(tp[:], tcs[:, h * 128:(h + 1) * 128],
                                         dg[:], start=True, stop=False)
                        nc.tensor.matmul(tp[:], tsn[:, h * 128:(h + 1) * 128],
                                         dg[:], start=False, stop=True)
                        if h in (2, 3):
                            nc.scalar.copy(qT[:, h, si * 128:(si + 1) * 128], tp[:])
                        else:
                            nc.vector.tensor_copy(qT[:, h, si * 128:(si + 1) * 128], tp[:])
                    tpk = psR.tile([128, 128], F32, tag="r")
                    nc.tensor.matmul(tpk[:], kcs[:], ident[:], start=True, stop=False)
                    nc.tensor.matmul(tpk[:], ksn[:], ident[:], start=False, stop=True)
                    nc.scalar.copy(kT[:, si * 128:(si + 1) * 128], tpk[:])
                    nc.scalar.copy(v_nat[:, si, :], kv_ps[:, 128:256])

                # ---------- attention for q-slice t ----------
                yt_t = ytp.tile([128, NH, 512], BF16, tag="yt")
                nblk = 4 * t + 4
                for hp in (2, 0):
                    o_ps = {}
                    rs_ps = {}
                    for h in (hp, hp + 1):
                        o_ps[h] = psO.tile([128, 512], F32, tag="o", name=f"o_{t}_{h}")
                        rs_ps[h] = psR.tile([1, 512], F32, tag="r", name=f"rs_{t}_{h}")
                    for j in range(nblk):
                        off = j - 4 * t
                        lo = max(off, 0) * 128
                        prb = {}
                        for h in (hp, hp + 1):
                            sc = psBig.tile([128, 512], F32, tag="bank",
                                            name=f"sc_{t}_{h}_{j}")
                            nc.tensor.matmul(
                                sc[:, lo:512],
                                kT[:, j * 128:(j + 1) * 128],
                                qT[:, h, t * 512 + lo:(t + 1) * 512],
                                start=True, stop=True)
                            pb = probsp.tile([128, 512], BF16, tag="probs",
                                             name=f"prb_{t}_{h}_{j}")
                            nc.scalar.activation(pb[:, lo:512], sc[:, lo:512],
                                                 AF.Exp, scale=rnk_all[:, j:j + 1])
                            if off >= 0:
                                nc.vector.tensor_tensor(
                                    pb[:, lo:lo + 128], pb[:, lo:lo + 128],
                                    tri[:], AluOpType.mult)
                            prb[h] = pb
                        for h in (hp, hp + 1):
                            nc.tensor.matmul(
                                o_ps[h][:, lo:512], v_nat[:, j, :], prb[h][:, lo:512],
                                start=(j == 0), stop=(j == nblk - 1),
                                skip_group_check=True)
                            nc.tensor.matmul(
                                rs_ps[h][:, lo:512], ones[:], prb[h][:, lo:512],
                                start=(j == 0), stop=(j == nblk - 1),
                                skip_group_check=True)
                        if j == 0:
                            flush()

                    def make_ep(o_ps=o_ps, rs_ps=rs_ps, yt_t=yt_t, hp=hp):
                        def ep():
                            for h in (hp, hp + 1):
                                nc.scalar.copy(yt_t[:, h, :], o_ps[h][:])
                                rr = normp.tile([1, 512], F32, tag="rr")
                                nc.vector.reciprocal_approx_fast(
                                    out=rr[:], in_=rs_ps[h][:])
                                rbc = normp.tile([128, 512], F32, tag="rbc")
                                nc.gpsimd.partition_broadcast(rbc[:], rr[:])
                                nc.vector.tensor_tensor(
                                    yt_t[:, h, :], yt_t[:, h, :], rbc[:],
                                    AluOpType.mult)
                        return ep
                    pending[0] = make_ep()

                # ---------- queue projection of this q-slice ----------
                emit_proj_groups(len(proj_queue))
                flush()
                queue_proj(yt_t, t)
                yt_prev[0] = yt_t

            emit_proj_groups(len(proj_queue))

    nc.compile()
    return nc


def _host_inputs(x, Wq, Wk, Wv, Wproj, q_gain):
    x = np.asarray(x, dtype=np.float32)
    Wq = np.asarray(Wq, dtype=np.float32)
    Wk = np.asarray(Wk, dtype=np.float32)
    Wv = np.asarray(Wv, dtype=np.float32)
    Wproj = np.asarray(Wproj, dtype=np.float32)
    q_gain = np.asarray(q_gain, dtype=np.float32)

    inv = (1.0 / ROPE_BASE ** (np.arange(0, HD, 2, dtype=np.float32) / HD)).astype(np.float32)
    ang = np.outer(np.arange(S, dtype=np.float32), inv)
    cos = np.cos(ang).astype(np.float32)
    sin = np.sin(ang).astype(np.float32)
    cos2 = np.concatenate([cos, cos], 1).reshape(NT, 128, HD).transpose(1, 0, 2)
    sin2 = np.concatenate([sin, -sin], 1).reshape(NT, 128, HD).transpose(1, 0, 2)

    kk = np.arange(128)[:, None]
    qq = np.arange(128)[None, :]
    tri = (kk <= qq).astype(BF)
    ident = np.eye(128, dtype=BF)
    ones = np.ones((128, 1), dtype=BF)

    in_maps = []
    for cid in range(8):
        b, g = cid // 4, cid % 4
        # x[b].T [D, S] -> [128, NT, NC_, 128]: (c%128, s//128, c//128, s%128)
        xts = np.ascontiguousarray(
            x[b].T.reshape(NC_, 128, NT, 128).transpose(1, 2, 0, 3)).astype(BF)
        wq = Wq[g * 512:(g + 1) * 512, :].T            # [D, 512]
        wk = Wk[g * 128:(g + 1) * 128, :].T            # [D, 128]
        wv = Wv[g * 128:(g + 1) * 128, :].T
        wqkv = np.concatenate([wq, wk, wv], 1)         # [D, 768]
        wpp = Wproj[:, g * 512:(g + 1) * 512].T        # [512, D]
        in_maps.append({
            "XTS": xts,
            "WQKV": np.ascontiguousarray(
                wqkv.reshape(NC_, 128, 768).transpose(1, 0, 2)).astype(BF),
            "WP": np.ascontiguousarray(
                wpp.reshape(NH, 128, D).transpose(1, 0, 2)).astype(BF),
            "COS2": np.ascontiguousarray(cos2).astype(BF),
            "SIN2": np.ascontiguousarray(sin2).astype(BF),
            "GSM": (q_gain[g * 4:(g + 1) * 4] * SM).reshape(1, NH).astype(np.float32),
            "IDENT": ident, "ONES": ones, "TRI": tri,
        })
    return in_maps


def _get_prog():
    global _PROG
    if _PROG is None:
        _PROG = _build_program()
    return _PROG


def kernel(x, Wq, Wk, Wv, Wproj, q_gain, _trace=False, _tmpdir=None):
    from concourse.bass_utils import run_bass_kernel_spmd
    nc = _get_prog()
    in_maps = _host_inputs(x, Wq, Wk, Wv, Wproj, q_gain)
    kwargs = {}
    if _tmpdir is not None:
        os.makedirs(_tmpdir, exist_ok=True)
        kwargs["tmpdir"] = _tmpdir
    res = run_bass_kernel_spmd(nc, in_maps, list(range(8)), trace=_trace, **kwargs)
    y = np.empty((B, S, D), dtype=np.float32)
    for b in range(B):
        acc = res.results[4 * b]["Y"].astype(np.float32)
        for g in range(1, 4):
            acc = acc + res.results[4 * b + g]["Y"].astype(np.float32)
        y[b] = acc
    if _trace:
        kernel._last_result = res
    return y


# revision 22
# speedup vs baseline: 1.1987x; 1.0073x over previous
"""Causal GQA self-attention (B=2, S=2048, D=2048, H=16, KV=4) on 8 TRN2 cores.

Sharding: core = (b, g) with b = batch (2) x g = kv-head group (4).
Each core computes 4 q-heads / 1 kv-head for one batch and a partial
projection output [S, D]; host sums the 4 group partials per batch.

v3: all-bf16 matmuls, per-t merged loop with proj of t-1 interleaved
after attention t, host-preswizzled x (contiguous DMA), q-norm via DVE
bn_stats, q norm-scale folded into the transpose as a diag matmul,
k norm-scale folded into exp's per-partition scale, rope add folded
into psum-accumulated transpose pair, proj written psum->DRAM directly.
"""
import os
import sys

if '/opt/trn_rl_repo' not in sys.path:
    sys.path.insert(0, '/opt/trn_rl_repo')

import numpy as np
import ml_dtypes

BF = ml_dtypes.bfloat16

B, S, D = 2, 2048, 2048
NH_TOT, NKV_TOT, HD = 16, 4, 128
NH = 4                 # q heads per core
NT = S // 128          # 16 s-tiles
NC_ = D // 128         # 16 c-tiles
T = 4                  # q-slices of 512
SM = 1.0 / np.sqrt(HD)
EPS = float(np.finfo(np.float32).eps)
ROPE_BASE = 10000.0
HORD = (2, 3, 0, 1)    # head emission order (attention runs pair hp=2 first)

_PROG = None


def _build_program():
    import concourse.bass as bass
    import concourse.mybir as mybir
    import concourse.tile as tile
    from concourse import bacc
    from concourse.alu_op_type import AluOpType

    F32 = mybir.dt.float32
    BF16 = mybir.dt.bfloat16
    AF = mybir.ActivationFunctionType

    nc = bacc.Bacc("TRN2", target_bir_lowering=False, debug=False)

    XTS = nc.dram_tensor("XTS", [128, NT, NC_, 128], BF16, kind="ExternalInput")
    WQKV = nc.dram_tensor("WQKV", [128, NC_, 768], BF16, kind="ExternalInput")
    WP = nc.dram_tensor("WP", [128, NH, D], BF16, kind="ExternalInput")
    COS2 = nc.dram_tensor("COS2", [128, NT, HD], BF16, kind="ExternalInput")
    SIN2 = nc.dram_tensor("SIN2", [128, NT, HD], BF16, kind="ExternalInput")
    GSM = nc.dram_tensor("GSM", [1, NH], F32, kind="ExternalInput")
    IDENT = nc.dram_tensor("IDENT", [128, 128], BF16, kind="ExternalInput")
    ONES = nc.dram_tensor("ONES", [128, 1], BF16, kind="ExternalInput")
    TRI = nc.dram_tensor("TRI", [128, 128], BF16, kind="ExternalInput")
    Y = nc.dram_tensor("Y", [S, D], BF16, kind="ExternalOutput")

    with tile.TileContext(nc) as tc:
        with (
            tc.tile_pool(name="const", bufs=1) as const,
            tc.tile_pool(name="w", bufs=8) as wpool,
            tc.tile_pool(name="wp", bufs=1) as wppool,
            tc.tile_pool(name="stream", bufs=4) as stream,
            tc.tile_pool(name="small", bufs=3) as small,
            tc.tile_pool(name="rope", bufs=3) as ropep,
            tc.tile_pool(name="diag", bufs=6) as diagp,
            tc.tile_pool(name="big", bufs=1) as big,
            tc.tile_pool(name="yt", bufs=2) as ytp,
            tc.tile_pool(name="probs", bufs=6) as probsp,
            tc.tile_pool(name="norm", bufs=2) as normp,
            tc.tile_pool(name="outsb", bufs=4) as outsb,
            tc.tile_pool(name="psBig", bufs=4, space="PSUM") as psBig,   # 4 banks
            tc.tile_pool(name="psO", bufs=2, space="PSUM") as psO,       # 2 banks
            tc.tile_pool(name="psR", bufs=2, space="PSUM") as psR,       # 2 banks
        ):
            ident = const.tile([128, 128], BF16)
            nc.sync.dma_start(ident[:], IDENT[:])
            ones = const.tile([128, 1], BF16)
            nc.sync.dma_start(ones[:], ONES[:])
            tri = const.tile([128, 128], BF16)
            nc.sync.dma_start(tri[:], TRI[:])
            gsm = const.tile([1, NH], F32)
            nc.sync.dma_start(gsm[:], GSM[:])
            gsm_bc = const.tile([128, NH], F32)
            nc.gpsimd.partition_broadcast(gsm_bc[:], gsm[:])

            cos2 = const.tile([128, NT, HD], BF16)
            nc.gpsimd.dma_start(cos2[:], COS2[:])
            sin2 = const.tile([128, NT, HD], BF16)
            nc.gpsimd.dma_start(sin2[:], SIN2[:])

            wqkv = []
            for c2 in range(8):
                wt = wpool.tile([128, 2, 768], BF16, tag="w")
                eng = nc.scalar if c2 % 2 == 0 else nc.gpsimd
                eng.dma_start(wt[:], WQKV[:, 2 * c2:2 * c2 + 2, :])
                wqkv.append(wt)

            wp = wppool.tile([128, NH, D], BF16)
            nc.gpsimd.dma_start(wp[:], WP[:])

            qT = big.tile([128, NH, S], BF16)
            kT = big.tile([128, S], BF16)
            v_nat = big.tile([128, NT, HD], BF16)
            rnk_all = big.tile([128, NT], F32)

            pending = [None]

            def flush():
                if pending[0] is not None:
                    pending[0]()
                    pending[0] = None

            yt_prev = [None]
            dmaq = [0]
            proj_queue = []

            def one_proj_group(yt_q, si, sl, dq):
                pj = psBig.tile([128, 512], F32, tag="bank",
                                name=f"pj_{si}_{dq}")
                for hi, h in enumerate(HORD):
                    nc.tensor.matmul(
                        pj[:], yt_q[:, h, sl * 128:(sl + 1) * 128],
                        wp[:, h, dq * 512:(dq + 1) * 512],
                        start=(hi == 0), stop=(hi == NH - 1),
                        skip_group_check=True)
                ev = outsb.tile([128, 512], BF16, tag="ev")
                if dq % 2 == 0:
                    nc.scalar.copy(ev[:], pj[:])
                else:
                    nc.vector.tensor_copy(ev[:], pj[:])
                eng = (nc.scalar, nc.sync)[dmaq[0] % 2]
                dmaq[0] += 1
                eng.dma_start(
                    Y[si * 128:(si + 1) * 128,
                      dq * 512:(dq + 1) * 512], ev[:])

            def queue_proj(yt_q, t_src):
                for si in range(4 * t_src, 4 * t_src + 4):
                    sl = si - 4 * t_src
                    for dq in range(4):
                        proj_queue.append((yt_q, si, sl, dq))

            def emit_proj_groups(n):
                for _ in range(min(n, len(proj_queue))):
                    one_proj_group(*proj_queue.pop(0))

            for t in range(T):
                # ---------- QKV for s-tiles 4t .. 4t+3 ----------
                for si in range(4 * t, 4 * t + 4):
                    xs = stream.tile([128, NC_, 128], BF16, tag="xs")
                    if si == 0:
                        for q_ in range(4):
                            nc.sync.dma_start(xs[:, 4 * q_:4 * q_ + 4, :],
                                              XTS[:, si, 4 * q_:4 * q_ + 4, :])
                    else:
                        nc.sync.dma_start(xs[:, 0:8, :], XTS[:, si, 0:8, :])
                        nc.sync.dma_start(xs[:, 8:16, :], XTS[:, si, 8:16, :])
                    q_ps = psBig.tile([128, 512], F32, tag="bank")
                    kv_ps = psBig.tile([128, 256], F32, tag="bank")
                    for ci in range(NC_):
                        nc.tensor.matmul(q_ps[:], xs[:, ci, :],
                                         wqkv[ci // 2][:, ci % 2, 0:512],
                                         start=(ci == 0), stop=(ci == NC_ - 1))
                        nc.tensor.matmul(kv_ps[:], xs[:, ci, :],
                                         wqkv[ci // 2][:, ci % 2, 512:768],
                                         start=(ci == 0), stop=(ci == NC_ - 1))
                    emit_proj_groups(4)

                    q4 = q_ps.rearrange("p (h d) -> p h d", h=NH)

                    # q sumsq: ACT Square (no accum) + one DVE reduce; k: ACT accum
                    ssq4 = small.tile([128, 8], F32, tag="ssq4")
                    scr_sq = small.tile([128, 512], F32, tag="scrsq")
                    nc.scalar.activation(scr_sq[:], q_ps[:], AF.Square)
                    nc.vector.tensor_reduce(
                        ssq4[:, 0:4], scr_sq.rearrange("p (h d) -> p h d", h=NH),
                        mybir.AxisListType.X, AluOpType.add)
                    scr = small.tile([128, 128], F32, tag="scr")
                    nc.scalar.activation(scr[:], kv_ps[:, 0:128], AF.Square,
                                         accum_out=ssq4[:, 4:5])
                    mn = small.tile([128, 8], F32, tag="mn")
                    nc.vector.tensor_scalar(mn[:, 0:5], ssq4[:, 0:5], 1.0 / HD, EPS,
                                            AluOpType.mult, AluOpType.add)
                    rt = small.tile([128, 8], F32, tag="rt")
                    nc.scalar.activation(rt[:, 0:5], mn[:, 0:5], AF.Sqrt)
                    rn = small.tile([128, 8], F32, tag="rn")
                    nc.vector.reciprocal_approx_fast(out=rn[:, 0:5], in_=rt[:, 0:5])
                    qsc = small.tile([128, 4], F32, tag="qsc")
                    nc.vector.tensor_tensor(qsc[:], rn[:, 0:4], gsm_bc[:],
                                            AluOpType.mult)
                    nc.vector.tensor_copy(rnk_all[:, si:si + 1], rn[:, 4:5])

                    # rope (merged over 4 q heads, unscaled; scale via diag)
                    tcs = ropep.tile([128, 512], BF16, tag="tcs")
                    tsn = ropep.tile([128, 512], BF16, tag="tsn")
                    t4c = tcs.rearrange("p (h d) -> p h d", h=NH)
                    t4s = tsn.rearrange("p (h d) -> p h d", h=NH)
                    cosb = cos2[:, si:si + 1, :].broadcast_to([128, NH, HD])
                    sina = sin2[:, si:si + 1, 0:64].broadcast_to([128, NH, 64])
                    sinb = sin2[:, si:si + 1, 64:128].broadcast_to([128, NH, 64])
                    nc.vector.tensor_tensor(t4c, q4, cosb, AluOpType.mult)
                    nc.vector.tensor_tensor(t4s[:, :, 0:64], q4[:, :, 64:128],
                                            sina, AluOpType.mult)
                    nc.vector.tensor_tensor(t4s[:, :, 64:128], q4[:, :, 0:64],
                                            sinb, AluOpType.mult)
                    # k rope on gpsimd (gpsimd can't read psum: stage via ACT)
                    knat = ropep.tile([128, 128], BF16, tag="knat")
                    nc.scalar.copy(knat[:], kv_ps[:, 0:128])
                    kcs = ropep.tile([128, 128], BF16, tag="kcs")
                    ksn = ropep.tile([128, 128], BF16, tag="ksn")
                    nc.gpsimd.tensor_tensor(kcs[:], knat[:],
                                            cos2[:, si, :], AluOpType.mult)
                    nc.gpsimd.tensor_tensor(ksn[:, 0:64], knat[:, 64:128],
                                            sin2[:, si, 0:64], AluOpType.mult)
                    nc.gpsimd.tensor_tensor(ksn[:, 64:128], knat[:, 0:64],
                                            sin2[:, si, 64:128], AluOpType.mult)

                    # diag(qsc_h) built on gpsimd; transpose-with-scale on PE
                    for h in HORD:
                        dg = diagp.tile([128, 128], BF16, tag="dg")
                        nc.vector.tensor_scalar(
                            dg[:], ident[:], qsc[:, h:h + 1], None,
                            AluOpType.mult)
                        tp = psR.tile([128, 128], F32, tag="r")
                        nc.tensor.matmul(tp[:], tcs[:, h * 128:(h + 1) * 128],
                                         dg[:], start=True, stop=False)
                        nc.tensor.matmul(tp[:], tsn[:, h * 128:(h + 1) * 128],
                                         dg[:], start=False, stop=True)
                        if h in (2, 3):
                            nc.scalar.copy(qT[:, h, si * 128:(si + 1) * 128], tp[:])
                        else:
                            nc.vector.tensor_copy(qT[:, h, si * 128:(si + 1) * 128], tp[:])
                    tpk = psR.tile([128, 128], F32, tag="r")
                    nc.tensor.matmul(tpk[:], kcs[:], ident[:], start=True, stop=False)
                    nc.tensor.matmul(tpk[:], ksn[:], ident[:], start=False, stop=True)
                    nc.scalar.copy(kT[:, si * 128:(si + 1) * 128], tpk[:])
                    nc.scalar.copy(v_nat[:, si, :], kv_ps[:, 128:256])

                # ---------- attention for q-slice t ----------
                yt_t = ytp.tile([128, NH, 512], BF16, tag="yt")
                nblk = 4 * t + 4
                for hp in (2, 0):
                    o_ps = {}
                    rs_ps = {}
                    for h in (hp, hp + 1):
                        o_ps[h] = psO.tile([128, 512], F32, tag="o", name=f"o_{t}_{h}")
                        rs_ps[h] = psR.tile([1, 512], F32, tag="r", name=f"rs_{t}_{h}")
                    for j in range(nblk):
                        off = j - 4 * t
                        lo = max(off, 0) * 128
                        prb = {}
                        for h in (hp, hp + 1):
                            sc = psBig.tile([128, 512], F32, tag="bank",
                                            name=f"sc_{t}_{h}_{j}")
                            nc.tensor.matmul(
                                sc[:, lo:512],
                                kT[:, j * 128:(j + 1) * 128],
                                qT[:, h, t * 512 + lo:(t + 1) * 512],
                                start=True, stop=True)
                            pb = probsp.tile([128, 512], BF16, tag="probs",
                                             name=f"prb_{t}_{h}_{j}")
                            nc.scalar.activation(pb[:, lo:512], sc[:, lo:512],
                                                 AF.Exp, scale=rnk_all[:, j:j + 1])
                            if off >= 0:
                                nc.vector.tensor_tensor(
                                    pb[:, lo:lo + 128], pb[:, lo:lo + 128],
                                    tri[:], AluOpType.mult)
                            prb[h] = pb
                        for h in (hp, hp + 1):
                            nc.tensor.matmul(
                                o_ps[h][:, lo:512], v_nat[:, j, :], prb[h][:, lo:512],
                                start=(j == 0), stop=(j == nblk - 1),
                                skip_group_check=True)
                            nc.tensor.matmul(
                                rs_ps[h][:, lo:512], ones[:], prb[h][:, lo:512],
                                start=(j == 0), stop=(j == nblk - 1),
                                skip_group_check=True)
                        if j == 0:
                            flush()

                    def make_ep(o_ps=o_ps, rs_ps=rs_ps, yt_t=yt_t, hp=hp):
                        def ep():
                            for h in (hp, hp + 1):
                                nc.scalar.copy(yt_t[:, h, :], o_ps[h][:])
                                rr = normp.tile([1, 512], F32, tag="rr")
                                nc.vector.reciprocal_approx_fast(
                                    out=rr[:], in_=rs_ps[h][:])
                                rbc = normp.tile([128, 512], F32, tag="rbc")
                                nc.gpsimd.partition_broadcast(rbc[:], rr[:])
                                nc.vector.tensor_tensor(
                                    yt_t[:, h, :], yt_t[:, h, :], rbc[:],
                                    AluOpType.mult)
                        return ep
                    pending[0] = make_ep()

                # ---------- queue projection of this q-slice ----------
                emit_proj_groups(len(proj_queue))
                flush()
                queue_proj(yt_t, t)
                yt_prev[0] = yt_t

            emit_proj_groups(len(proj_queue))

    nc.compile()
    return nc


def _host_inputs(x, Wq, Wk, Wv, Wproj, q_gain):
    x = np.asarray(x, dtype=np.float32)
    Wq = np.asarray(Wq, dtype=np.float32)
    Wk = np.asarray(Wk, dtype=np.float32)
    Wv = np.asarray(Wv, dtype=np.float32)
    Wproj = np.asarray(Wproj, dtype=np.float32)
    q_gain = np.asarray(q_gain, dtype=np.float32)

    inv = (1.0 / ROPE_BASE ** (np.arange(0, HD, 2, dtype=np.float32) / HD)).astype(np.float32)
    ang = np.outer(np.arange(S, dtype=np.float32), inv)
    cos = np.cos(ang).astype(np.float32)
    sin = np.sin(ang).astype(np.float32)
    cos2 = np.concatenate([cos, cos], 1).reshape(NT, 128, HD).transpose(1, 0, 2)
    sin2 = np.concatenate([sin, -sin], 1).reshape(NT, 128, HD).transpose(1, 0, 2)

    kk = np.arange(128)[:, None]
    qq = np.arange(128)[None, :]
    tri = (kk <= qq).astype(BF)
    ident = np.eye(128, dtype=BF)
    ones = np.ones((128, 1), dtype=BF)

    in_maps = []
    for cid in range(8):
        b, g = cid // 4, cid % 4
        # x[b].T [D, S] -> [128, NT, NC_, 128]: (c%128, s//128, c//128, s%128)
        xts = np.ascontiguousarray(
            x[b].T.reshape(NC_, 128, NT, 128).transpose(1, 2, 0, 3)).astype(BF)
        wq = Wq[g * 512:(g + 1) * 512, :].T            # [D, 512]
        wk = Wk[g * 128:(g + 1) * 128, :].T            # [D, 128]
        wv = Wv[g * 128:(g + 1) * 128, :].T
        wqkv = np.concatenate([wq, wk, wv], 1)         # [D, 768]
        wpp = Wproj[:, g * 512:(g + 1) * 512].T        # [512, D]
        in_maps.append({
            "XTS": xts,
            "WQKV": np.ascontiguousarray(
                wqkv.reshape(NC_, 128, 768).transpose(1, 0, 2)).astype(BF),
            "WP": np.ascontiguousarray(
                wpp.reshape(NH, 128, D).transpose(1, 0, 2)).astype(BF),
            "COS2": np.ascontiguousarray(cos2).astype(BF),
            "SIN2": np.ascontiguousarray(sin2).astype(BF),
            "GSM": (q_gain[g * 4:(g + 1) * 4] * SM).reshape(1, NH).astype(np.float32),
            "IDENT": ident, "ONES": ones, "TRI": tri,
        })
    return in_maps


def _get_prog():
    global _PROG
    if _PROG is None:
        _PROG = _build_program()
    return _PROG


def kernel(x, Wq, Wk, Wv, Wproj, q_gain, _trace=False, _tmpdir=None):
    from concourse.bass_utils import run_bass_kernel_spmd
    nc = _get_prog()
    in_maps = _host_inputs(x, Wq, Wk, Wv, Wproj, q_gain)
    kwargs = {}
    if _tmpdir is not None:
        os.makedirs(_tmpdir, exist_ok=True)
        kwargs["tmpdir"] = _tmpdir
    res = run_bass_kernel_spmd(nc, in_maps, list(range(8)), trace=_trace, **kwargs)
    y = np.empty((B, S, D), dtype=np.float32)
    for b in range(B):
        acc = res.results[4 * b]["Y"].astype(np.float32)
        for g in range(1, 4):
            acc = acc + res.results[4 * b + g]["Y"].astype(np.float32)
        y[b] = acc
    if _trace:
        kernel._last_result = res
    return y
